# revision 17
# baseline (speedup 1.0000x reference)
"""Trainium2 Bass kernel for nn_GAT_TRANSFORMER (retrieval_knn).

Computes, per batch row b (bsn=300000, n=7 neighbours, m=16 cams):
  prior_dir = l2norm(feat[:, :3]);  cos = prior_dir @ cam^T
  prob = softmax(cos); idx = argmax(cos); match_cos = max(cos)
  valid = match_cos > 0.99
  pos = valid ? dis * cam[idx] : prior_pos
  cov = valid ? clip((1-match_cos)*100, 0.01, 10) : 10
  indices = valid ? idx : -1;  scores = full(-inf)

Sharding: pure data-parallel over the bsn axis across 8 NeuronCores.
Layout on core: batch rows on the 128 SBUF partitions, R rows per
partition per tile; all per-row structure ([7,16] etc.) lives in the
free dimension and per-n broadcasts are done with zero-stride APs.
Work is split across the DVE (reduces + part of the elementwise),
GPSIMD (part of the elementwise) and ACT (exp/square/copies).
"""
import numpy as np

import concourse.bass as bass
import concourse.bacc as bacc
import concourse.mybir as mybir
import concourse.tile as tile
from concourse.bass_utils import run_bass_kernel_spmd

NN = 7
MM = 16
BSN = 300000
NCORES = 8
ROWS_CORE = BSN // NCORES          # 37500
P = 128
MAX_COV = 10.0

f32 = mybir.dt.float32
Alu = mybir.AluOpType
Act = mybir.ActivationFunctionType
AxX = mybir.AxisListType.X

_CACHE = {}

# Default geometry: R rows per partition per tile, NT tiles per core.
R_DEF = 20
NT_DEF = 15

# Which engine runs each splittable elementwise op ("v" = DVE, "g" = GPSIMD).
CFG_DEF = {
    # NOTE: the public neuronxcc ISA rejects TensorTensor on the Pool
    # engine, so GPSIMD cannot run elementwise ops here; everything
    # splittable stays on the DVE.
    "t0": "v", "t1": "v", "t2": "v",     # cos partial products
    "add1": "v", "add2": "v",            # cos accumulation
    "prob": "v",                         # prob = ex * rsum
    "eq": "v", "eqw": "v", "oh": "v",    # argmax chain
    "tmp": "v",                          # one-hot * camT products
    "posml": "v", "possub": "v", "posmv": "v", "posadd": "v",
    "trees": "",                         # (gps trees unavailable)
    "newton": 1,                         # rsqrt Newton iterations
    "pospred": 1,                        # pos blend via copy_predicated
}


def _eng(nc, cfg, key):
    return nc.gpsimd if cfg.get(key, "v") == "g" else nc.vector


def _gps_tree(nc, pool, src_ap, out, groups, op, tag):
    """Grouped reduce over innermost 16 via gpsimd halving adds/maxes."""
    cur = src_ap
    width = 16
    while width > 2:
        half = width // 2
        nxt = pool.tile([P, groups, half], f32, tag=f"{tag}{half}")
        nc.gpsimd.tensor_tensor(nxt, cur[:, :, 0:half], cur[:, :, half:width], op)
        cur = nxt
        width = half
    nc.gpsimd.tensor_tensor(out, cur[:, :, 0:1].squeeze(2), cur[:, :, 1:2].squeeze(2), op)


def _build_module(reps=1, cfg=None, rr=R_DEF, nt=NT_DEF):
    cfg = dict(CFG_DEF, **(cfg or {}))
    R, NT = rr, nt
    rows_pad = P * R * NT
    assert rows_pad >= ROWS_CORE

    nc = bacc.Bacc("TRN2", target_bir_lowering=False)

    feat = nc.dram_tensor("feat", [rows_pad * NN, 8], f32, kind="ExternalInput")
    cam = nc.dram_tensor("cam", [rows_pad * MM, 3], f32, kind="ExternalInput")
    wdesc = nc.dram_tensor("wdesc", [MM], f32, kind="ExternalInput")   # 15..0
    iota = nc.dram_tensor("iota", [MM], f32, kind="ExternalInput")     # 0..15

    prob_o = nc.dram_tensor("prob_o", [rows_pad, NN, MM], f32, kind="ExternalOutput")
    pos_o = nc.dram_tensor("pos_o", [rows_pad, NN, 3], f32, kind="ExternalOutput")
    cov_o = nc.dram_tensor("cov_o", [rows_pad, NN], f32, kind="ExternalOutput")
    ind_o = nc.dram_tensor("ind_o", [rows_pad, NN], mybir.dt.int32, kind="ExternalOutput")
    sco_o = nc.dram_tensor("sco_o", [rows_pad * (NN + 1) * (MM + 1)], f32, kind="ExternalOutput")

    feat_v = feat.rearrange("(t p r n) c -> t p (r n c)", t=NT, p=P, r=R, n=NN)
    cam_v = cam.rearrange("(t p r m) c -> t p (r m c)", t=NT, p=P, r=R, m=MM)
    prob_v = prob_o.rearrange("(t p r) n m -> t p (r n m)", t=NT, p=P, r=R)
    pos_v = pos_o.rearrange("(t p r) n d -> t p (r n d)", t=NT, p=P, r=R)
    cov_v = cov_o.rearrange("(t p r) n -> t p (r n)", t=NT, p=P, r=R)
    ind_v = ind_o.rearrange("(t p r) n -> t p (r n)", t=NT, p=P, r=R)

    sco_per_p = rows_pad * (NN + 1) * (MM + 1) // P
    sco_chunk = max(c for c in range(1, 4097) if sco_per_p % c == 0)
    sco_rep = sco_per_p // sco_chunk
    sco_v = sco_o.rearrange("(p f) -> p f", p=P)

    with tile.TileContext(nc) as tc:
        with tc.tile_pool(name="consts", bufs=1) as consts, \
             tc.tile_pool(name="io", bufs=3) as io, \
             tc.tile_pool(name="work", bufs=2) as work:

            wt = consts.tile([P, MM], f32)
            nc.sync.dma_start(out=wt, in_=wdesc[None, :].partition_broadcast(P))
            it = consts.tile([P, MM], f32)
            nc.sync.dma_start(out=it, in_=iota[None, :].partition_broadcast(P))

            # scores: constant -inf, written straight from a broadcast tile
            ninf = consts.tile([P, sco_chunk], f32)
            nc.vector.memset(ninf, float("-inf"))
            for j in range(sco_rep):
                nc.sync.dma_start(
                    out=sco_v[:, j * sco_chunk:(j + 1) * sco_chunk], in_=ninf)

            for rep in range(reps):
                for t in range(NT):
                    ft = io.tile([P, R, NN, 8], f32, tag="ft")
                    nc.sync.dma_start(out=ft, in_=feat_v[t])
                    ct = io.tile([P, R, MM, 3], f32, tag="ct")
                    nc.sync.dma_start(out=ct, in_=cam_v[t])

                    pos_ap = ft[:, :, :, 0:3]
                    dis_ap = ft[:, :, :, 7:8]

                    # camT: d-major cam copy for the match contraction (ACT)
                    camT = work.tile([P, R, 3, MM], f32, tag="camT")
                    nc.scalar.activation(out=camT, in_=ct.transpose([0, 1, 3, 2]), func=Act.Copy)

                    # ---- rnorm = 1/max(|pos|,1e-12): ACT rsqrt estimate
                    # + Newton refinement (cov amplifies mx error x100, so
                    # the ~1e-5 ACT spline must be refined to fp32 level;
                    # one step squares the error) ----
                    sq = work.tile([P, R, NN, 3], f32, tag="sq")
                    nc.scalar.activation(out=sq, in_=pos_ap, func=Act.Square)
                    nrm = work.tile([P, R * NN], f32, tag="nrm")
                    nc.vector.reduce_sum(out=nrm, in_=sq.rearrange("p r n d -> p (r n) d"), axis=AxX)
                    nc.vector.tensor_scalar_max(nrm, nrm, 1e-24)
                    rno = work.tile([P, R * NN], f32, tag="rno")
                    nc.scalar.activation(out=rno, in_=nrm, func=Act.Abs_reciprocal_sqrt)
                    nwt = work.tile([P, R * NN], f32, tag="v10")
                    for _ in range(cfg.get("newton", 2)):
                        nc.vector.tensor_mul(nwt, rno, rno)
                        # nwt = (-0.5 * r^2) * norm2
                        nc.vector.scalar_tensor_tensor(nwt, nwt, -0.5, nrm, Alu.mult, Alu.mult)
                        # r = (1.5 + nwt) * r
                        nc.vector.scalar_tensor_tensor(rno, nwt, 1.5, rno, Alu.add, Alu.mult)
                    dirt = work.tile([P, R, NN, 3], f32, tag="sq")
                    rno_b = rno.rearrange("p (r n) -> p r n", r=R)[:, :, :, None] \
                               .broadcast_to((P, R, NN, 3))
                    nc.vector.tensor_mul(dirt, pos_ap, rno_b)

                    # ---- cos[n,m] = sum_d dir[n,d]*cam[m,d] ----
                    ts = []
                    for d in range(3):
                        td = work.tile([P, R, NN, MM], f32, tag=f"t{d}")
                        dir_d = dirt[:, :, :, d:d + 1].broadcast_to((P, R, NN, MM))
                        cam_d = ct[:, :, None, :, d].broadcast_to((P, R, NN, MM))
                        _eng(nc, cfg, f"t{d}").tensor_mul(td, dir_d, cam_d)
                        ts.append(td)
                    cos = ts[0]
                    _eng(nc, cfg, "add1").tensor_add(cos, cos, ts[1])
                    _eng(nc, cfg, "add2").tensor_add(cos, cos, ts[2])

                    # ---- prob = exp(cos) / sum(exp(cos)) ----
                    prob = io.tile([P, R, NN, MM], f32, tag="prob")
                    nc.scalar.activation(out=prob, in_=cos, func=Act.Exp)
                    sume = work.tile([P, R * NN], f32, tag="sume")
                    nc.vector.reduce_sum(out=sume, in_=prob.rearrange("p r n m -> p (r n) m"), axis=AxX)
                    rsum = work.tile([P, R * NN], f32, tag="rsum")
                    nc.vector.reciprocal(rsum, sume)
                    rsum_b = rsum.rearrange("p (r n) -> p r n", r=R)[:, :, :, None] \
                                 .broadcast_to((P, R, NN, MM))
                    _eng(nc, cfg, "prob").tensor_mul(prob, prob, rsum_b)
                    nc.sync.dma_start(out=prob_v[t], in_=prob)

                    # ---- argmax (first-max, tie-exact) ----
                    mx = work.tile([P, R * NN], f32, tag="mx")
                    nc.vector.reduce_max(out=mx, in_=cos.rearrange("p r n m -> p (r n) m"), axis=AxX)
                    eq = work.tile([P, R, NN, MM], f32, tag="eq")
                    mx_b = mx.rearrange("p (r n) -> p r n", r=R)[:, :, :, None] \
                             .broadcast_to((P, R, NN, MM))
                    _eng(nc, cfg, "eq").tensor_tensor(eq, cos, mx_b, Alu.is_equal)
                    wt_b = wt[:, None, None, :].broadcast_to((P, R, NN, MM))
                    _eng(nc, cfg, "eqw").tensor_mul(eq, eq, wt_b)
                    wm = work.tile([P, R * NN], f32, tag="wm")
                    nc.vector.reduce_max(out=wm, in_=eq.rearrange("p r n m -> p (r n) m"), axis=AxX)
                    idxf = work.tile([P, R * NN], f32, tag="idxf")
                    nc.vector.tensor_scalar(idxf, wm, -1.0, 15.0, Alu.mult, Alu.add)

                    # ---- match_cam = sum_m onehot(idx)[m] * cam[m,:] ----
                    oh = work.tile([P, R, NN, MM], f32, tag="eqx")
                    it_b = it[:, None, None, :].broadcast_to((P, R, NN, MM))
                    idx_b = idxf.rearrange("p (r n) -> p r n", r=R)[:, :, :, None] \
                                .broadcast_to((P, R, NN, MM))
                    _eng(nc, cfg, "oh").tensor_tensor(oh, it_b, idx_b, Alu.is_equal)
                    match = work.tile([P, R, NN, 3], f32, tag="match")
                    for d in range(3):
                        tmpd = work.tile([P, R, NN, MM], f32, tag="tmpd")
                        camT_d = camT[:, :, None, d, :].broadcast_to((P, R, NN, MM))
                        _eng(nc, cfg, "tmp").tensor_mul(tmpd, oh, camT_d)
                        nc.vector.reduce_sum(
                            out=match[:, :, :, d],
                            in_=tmpd.rearrange("p r n m -> p (r n) m"), axis=AxX)

                    # ---- valid + blends ----
                    valid = work.tile([P, R * NN], f32, tag="valid")
                    nc.vector.tensor_single_scalar(valid, mx, 0.99, Alu.is_gt)

                    posx = io.tile([P, R, NN, 3], f32, tag="posx")
                    dis_b = dis_ap.broadcast_to((P, R, NN, 3))
                    if cfg.get("pospred"):
                        # pos = dis*match, overwritten with prior_pos where invalid
                        inval = work.tile([P, R * NN], mybir.dt.uint8, tag="nrm")
                        nc.vector.tensor_single_scalar(inval, mx, 0.99, Alu.is_le)
                        nc.vector.tensor_mul(posx, match, dis_b)
                        inval_b3 = inval[:, :, None].broadcast_to((P, R * NN, 3))
                        nc.vector.copy_predicated(
                            posx.rearrange("p r n d -> p (r n) d"), inval_b3,
                            pos_ap.rearrange("p r n d -> p (r n) d"))
                    else:
                        # pos = pp + valid*(dis*match - pp)
                        _eng(nc, cfg, "posml").tensor_mul(posx, match, dis_b)
                        _eng(nc, cfg, "possub").tensor_sub(posx, posx, pos_ap)
                        valid_b3 = valid.rearrange("p (r n) -> p r n", r=R)[:, :, :, None] \
                                        .broadcast_to((P, R, NN, 3))
                        _eng(nc, cfg, "posmv").tensor_mul(posx, posx, valid_b3)
                        _eng(nc, cfg, "posadd").tensor_add(posx, posx, pos_ap)
                    nc.sync.dma_start(out=pos_v[t], in_=posx)

                    # cov = ((1-mx)*100 clipped to [0.01,10]; 10 when invalid)
                    cov0 = work.tile([P, R * NN], f32, tag="cov0")
                    nc.vector.tensor_scalar(cov0, mx, -1.0, 1.0, Alu.mult, Alu.add)
                    nc.vector.tensor_scalar(cov0, cov0, 100.0, 0.01, Alu.mult, Alu.max)
                    v10 = work.tile([P, R * NN], f32, tag="v10")
                    nc.vector.tensor_scalar(v10, mx, 0.99, MAX_COV, Alu.is_le, Alu.mult)
                    cov = io.tile([P, R * NN], f32, tag="cov")
                    # cov = max(min(cov0, 10), v10)
                    nc.vector.scalar_tensor_tensor(cov, cov0, MAX_COV, v10, Alu.min, Alu.max)
                    nc.sync.dma_start(out=cov_v[t], in_=cov)

                    # indices = valid*(idx+1) - 1
                    indf = work.tile([P, R * NN], f32, tag="indf")
                    nc.vector.scalar_tensor_tensor(indf, idxf, 1.0, valid, Alu.add, Alu.mult)
                    nc.vector.tensor_scalar_sub(indf, indf, 1.0)
                    ind = io.tile([P, R * NN], mybir.dt.int32, tag="ind")
                    nc.vector.tensor_copy(ind, indf)
                    nc.sync.dma_start(out=ind_v[t], in_=ind)

    nc.compile()
    return nc, rows_pad


def _get_module():
    if "nc" not in _CACHE:
        _CACHE["nc"] = _build_module()
    return _CACHE["nc"]


def _make_in_maps(others_feat, others_cam, rows_pad):
    feat = np.ascontiguousarray(others_feat, dtype=np.float32).reshape(BSN, NN, 8)
    cam = np.ascontiguousarray(others_cam, dtype=np.float32).reshape(BSN, MM, 3)
    wdesc_np = np.arange(MM - 1, -1, -1, dtype=np.float32)
    iota_np = np.arange(MM, dtype=np.float32)
    in_maps = []
    for c in range(NCORES):
        fpad = np.zeros((rows_pad, NN, 8), dtype=np.float32)
        cpad = np.zeros((rows_pad, MM, 3), dtype=np.float32)
        fpad[:ROWS_CORE] = feat[c * ROWS_CORE:(c + 1) * ROWS_CORE]
        cpad[:ROWS_CORE] = cam[c * ROWS_CORE:(c + 1) * ROWS_CORE]
        in_maps.append({
            "feat": fpad.reshape(rows_pad * NN, 8),
            "cam": cpad.reshape(rows_pad * MM, 3),
            "wdesc": wdesc_np,
            "iota": iota_np,
        })
    return in_maps


def _gather(res_list, rows_pad):
    prob = np.concatenate([r["prob_o"][:ROWS_CORE] for r in res_list], axis=0)
    pos = np.concatenate([r["pos_o"][:ROWS_CORE] for r in res_list], axis=0)
    cov = np.concatenate([r["cov_o"][:ROWS_CORE, :, None] for r in res_list], axis=0)
    ind = np.concatenate([r["ind_o"][:ROWS_CORE, :, None] for r in res_list], axis=0)
    sco = np.concatenate(
        [r["sco_o"].reshape(rows_pad, NN + 1, MM + 1)[:ROWS_CORE] for r in res_list],
        axis=0)
    return (prob, pos, cov.astype(np.float32), sco, ind.astype(np.int32))


def run(others_feat, others_cam):
    nc, rows_pad = _get_module()
    in_maps = _make_in_maps(others_feat, others_cam, rows_pad)
    res = run_bass_kernel_spmd(nc, in_maps, core_ids=list(range(NCORES)))
    return _gather(res.results, rows_pad), res.exec_time_ns


def kernel(others_feat, others_cam):
    out, _ = run(others_feat, others_cam)
    return out


def _pjrt_exec(nc, in_maps, iters=5):
    """Mirror bass2jax's sharded execution without donation; return
    (per-core results, list of wall times for repeated executions)."""
    import jax
    from jax.sharding import Mesh, PartitionSpec, NamedSharding
    from jax.experimental.shard_map import shard_map
    import concourse.mybir as mb
    from concourse import bass2jax as b2j
    import time

    b2j.install_neuronx_cc_hook()
    partition_name = nc.partition_id_tensor.name if nc.partition_id_tensor else None
    in_names, out_names, out_avals, zero_outs = [], [], [], []
    for alloc in nc.m.functions[0].allocations:
        if not isinstance(alloc, mb.MemoryLocationSet):
            continue
        name = alloc.memorylocations[0].name
        if alloc.kind == "ExternalInput":
            if name != partition_name:
                in_names.append(name)
        elif alloc.kind == "ExternalOutput":
            out_names.append(name)
            shape = tuple(alloc.tensor_shape)
            dtype = mb.dt.np(alloc.dtype)
            out_avals.append(jax.core.ShapedArray(shape, dtype))
            zero_outs.append(np.zeros(shape, dtype))
    n_params = len(in_names)
    all_names = in_names + out_names
    if partition_name is not None:
        all_names = all_names + [partition_name]

    def _body(*args):
        operands = list(args)
        if partition_name is not None:
            operands.append(b2j.partition_id_tensor())
        outs = b2j._bass_exec_p.bind(
            *operands,
            out_avals=tuple(out_avals),
            in_names=tuple(all_names),
            out_names=tuple(out_names),
            lowering_input_output_aliases=(),
            sim_require_finite=True,
            sim_require_nnan=True,
            nc=nc,
        )
        return tuple(outs)

    devices = jax.devices()[:NCORES]
    mesh = Mesh(np.asarray(devices), ("core",))
    sh = NamedSharding(mesh, PartitionSpec("core"))
    in_specs = (PartitionSpec("core"),) * (n_params + len(out_names))
    out_specs = (PartitionSpec("core"),) * len(out_names)
    fn = jax.jit(shard_map(_body, mesh=mesh, in_specs=in_specs,
                           out_specs=out_specs, check_rep=False),
                 keep_unused=True)

    concat_in = [
        jax.device_put(
            np.concatenate([np.asarray(m[nm]) for m in in_maps], axis=0), sh)
        for nm in in_names
    ]
    concat_zeros = [
        jax.device_put(np.zeros((NCORES * z.shape[0], *z.shape[1:]), z.dtype), sh)
        for z in zero_outs
    ]

    out_arrs = fn(*concat_in, *concat_zeros)
    jax.block_until_ready(out_arrs)
    times = []
    for _ in range(iters):
        t0 = time.perf_counter()
        o = fn(*concat_in, *concat_zeros)
        jax.block_until_ready(o)
        times.append(time.perf_counter() - t0)

    res = [
        {name: np.asarray(out_arrs[i]).reshape(NCORES, *out_avals[i].shape)[c]
         for i, name in enumerate(out_names)}
        for c in range(NCORES)
    ]
    return res, times


def run_timed(others_feat, others_cam, iters=5):
    nc, rows_pad = _get_module()
    in_maps = _make_in_maps(others_feat, others_cam, rows_pad)
    res, times = _pjrt_exec(nc, in_maps, iters=iters)
    wall_ns = int(min(times) * 1e9)
    return _gather(res, rows_pad), wall_ns, times


def sim_exec_ns():
    """Cost-model (TimelineSim) per-core exec estimate — used when no
    hardware profiling (NTFF) is available in the container."""
    from concourse.timeline_sim import TimelineSim
    nc, _ = _get_module()
    ts = TimelineSim(nc, trace=False, no_exec=True)
    ts.simulate()
    return int(ts.time)


# revision 18
# speedup vs baseline: 1.0221x; 1.0221x over previous
"""Trainium2 Bass kernel for nn_GAT_TRANSFORMER (retrieval_knn).

Computes, per batch row b (bsn=300000, n=7 neighbours, m=16 cams):
  prior_dir = l2norm(feat[:, :3]);  cos = prior_dir @ cam^T
  prob = softmax(cos); idx = argmax(cos); match_cos = max(cos)
  valid = match_cos > 0.99
  pos = valid ? dis * cam[idx] : prior_pos
  cov = valid ? clip((1-match_cos)*100, 0.01, 10) : 10
  indices = valid ? idx : -1;  scores = full(-inf)

Sharding: pure data-parallel over the bsn axis across 8 NeuronCores.
Layout on core: batch rows on the 128 SBUF partitions, R rows per
partition per tile; all per-row structure ([7,16] etc.) lives in the
free dimension and per-n broadcasts are done with zero-stride APs.
Work is split across the DVE (reduces + part of the elementwise),
GPSIMD (part of the elementwise) and ACT (exp/square/copies).
"""
import numpy as np

import concourse.bass as bass
import concourse.bacc as bacc
import concourse.mybir as mybir
import concourse.tile as tile
from concourse.bass_utils import run_bass_kernel_spmd

NN = 7
MM = 16
BSN = 300000
NCORES = 8
ROWS_CORE = BSN // NCORES          # 37500
P = 128
MAX_COV = 10.0

f32 = mybir.dt.float32
Alu = mybir.AluOpType
Act = mybir.ActivationFunctionType
AxX = mybir.AxisListType.X

_CACHE = {}

# Default geometry: R rows per partition per tile, NT tiles per core.
R_DEF = 20
NT_DEF = 15

# Which engine runs each splittable elementwise op ("v" = DVE, "g" = GPSIMD).
CFG_DEF = {
    # NOTE: the public neuronxcc ISA rejects TensorTensor on the Pool
    # engine, so GPSIMD cannot run elementwise ops here; everything
    # splittable stays on the DVE.
    "t0": "v", "t1": "v", "t2": "v",     # cos partial products
    "add1": "v", "add2": "v",            # cos accumulation
    "prob": "v",                         # prob = ex * rsum
    "eq": "v", "eqw": "v", "oh": "v",    # argmax chain
    "tmp": "v",                          # one-hot * camT products
    "posml": "v", "possub": "v", "posmv": "v", "posadd": "v",
    "trees": "",                         # (gps trees unavailable)
    "newton": 1,                         # rsqrt Newton iterations
    "pospred": 1,                        # pos blend via copy_predicated
}


def _eng(nc, cfg, key):
    return nc.gpsimd if cfg.get(key, "v") == "g" else nc.vector


def _gps_tree(nc, pool, src_ap, out, groups, op, tag):
    """Grouped reduce over innermost 16 via gpsimd halving adds/maxes."""
    cur = src_ap
    width = 16
    while width > 2:
        half = width // 2
        nxt = pool.tile([P, groups, half], f32, tag=f"{tag}{half}")
        nc.gpsimd.tensor_tensor(nxt, cur[:, :, 0:half], cur[:, :, half:width], op)
        cur = nxt
        width = half
    nc.gpsimd.tensor_tensor(out, cur[:, :, 0:1].squeeze(2), cur[:, :, 1:2].squeeze(2), op)


def _build_module(reps=1, cfg=None, rr=R_DEF, nt=NT_DEF):
    cfg = dict(CFG_DEF, **(cfg or {}))
    R, NT = rr, nt
    rows_pad = P * R * NT
    assert rows_pad >= ROWS_CORE

    nc = bacc.Bacc("TRN2", target_bir_lowering=False)

    feat = nc.dram_tensor("feat", [rows_pad * NN, 8], f32, kind="ExternalInput")
    cam = nc.dram_tensor("cam", [rows_pad * MM, 3], f32, kind="ExternalInput")
    wdesc = nc.dram_tensor("wdesc", [MM], f32, kind="ExternalInput")   # 15..0
    iota = nc.dram_tensor("iota", [MM], f32, kind="ExternalInput")     # 0..15

    prob_o = nc.dram_tensor("prob_o", [rows_pad, NN, MM], f32, kind="ExternalOutput")
    pos_o = nc.dram_tensor("pos_o", [rows_pad, NN, 3], f32, kind="ExternalOutput")
    cov_o = nc.dram_tensor("cov_o", [rows_pad, NN], f32, kind="ExternalOutput")
    ind_o = nc.dram_tensor("ind_o", [rows_pad, NN], mybir.dt.int32, kind="ExternalOutput")
    sco_o = nc.dram_tensor("sco_o", [rows_pad * (NN + 1) * (MM + 1)], f32, kind="ExternalOutput")

    feat_v = feat.rearrange("(t p r n) c -> t p (r n c)", t=NT, p=P, r=R, n=NN)
    cam_v = cam.rearrange("(t p r m) c -> t p (r m c)", t=NT, p=P, r=R, m=MM)
    prob_v = prob_o.rearrange("(t p r) n m -> t p (r n m)", t=NT, p=P, r=R)
    pos_v = pos_o.rearrange("(t p r) n d -> t p (r n d)", t=NT, p=P, r=R)
    cov_v = cov_o.rearrange("(t p r) n -> t p (r n)", t=NT, p=P, r=R)
    ind_v = ind_o.rearrange("(t p r) n -> t p (r n)", t=NT, p=P, r=R)

    sco_per_p = rows_pad * (NN + 1) * (MM + 1) // P
    sco_chunk = max(c for c in range(1, 4097) if sco_per_p % c == 0)
    sco_rep = sco_per_p // sco_chunk
    sco_v = sco_o.rearrange("(p f) -> p f", p=P)

    with tile.TileContext(nc) as tc:
        with tc.tile_pool(name="consts", bufs=1) as consts, \
             tc.tile_pool(name="io", bufs=3) as io, \
             tc.tile_pool(name="work", bufs=2) as work:

            wt = consts.tile([P, MM], f32)
            nc.sync.dma_start(out=wt, in_=wdesc[None, :].partition_broadcast(P))
            wt16 = consts.tile([P, MM], mybir.dt.bfloat16)
            nc.scalar.activation(out=wt16, in_=wt, func=Act.Copy)
            it = consts.tile([P, MM], f32)
            nc.sync.dma_start(out=it, in_=iota[None, :].partition_broadcast(P))

            # scores: constant -inf, written straight from a broadcast tile
            ninf = consts.tile([P, sco_chunk], f32)
            nc.vector.memset(ninf, float("-inf"))
            for j in range(sco_rep):
                nc.sync.dma_start(
                    out=sco_v[:, j * sco_chunk:(j + 1) * sco_chunk], in_=ninf)

            for rep in range(reps):
                for t in range(NT):
                    ft = io.tile([P, R, NN, 8], f32, tag="ft")
                    nc.sync.dma_start(out=ft, in_=feat_v[t])
                    ct = io.tile([P, R, MM, 3], f32, tag="ct")
                    nc.sync.dma_start(out=ct, in_=cam_v[t])

                    pos_ap = ft[:, :, :, 0:3]
                    dis_ap = ft[:, :, :, 7:8]

                    # camT: d-major cam copy for the match contraction (ACT)
                    camT = work.tile([P, R, 3, MM], f32, tag="camT")
                    nc.scalar.activation(out=camT, in_=ct.transpose([0, 1, 3, 2]), func=Act.Copy)

                    # ---- rnorm = 1/max(|pos|,1e-12): ACT rsqrt estimate
                    # + Newton refinement (cov amplifies mx error x100, so
                    # the ~1e-5 ACT spline must be refined to fp32 level;
                    # one step squares the error) ----
                    sq = work.tile([P, R, NN, 3], f32, tag="sq")
                    nc.scalar.activation(out=sq, in_=pos_ap, func=Act.Square)
                    nrm = work.tile([P, R * NN], f32, tag="nrm")
                    nc.vector.reduce_sum(out=nrm, in_=sq.rearrange("p r n d -> p (r n) d"), axis=AxX)
                    nc.vector.tensor_scalar_max(nrm, nrm, 1e-24)
                    rno = work.tile([P, R * NN], f32, tag="rno")
                    nc.scalar.activation(out=rno, in_=nrm, func=Act.Abs_reciprocal_sqrt)
                    nwt = work.tile([P, R * NN], f32, tag="v10")
                    for _ in range(cfg.get("newton", 2)):
                        nc.vector.tensor_mul(nwt, rno, rno)
                        # nwt = (-0.5 * r^2) * norm2
                        nc.vector.scalar_tensor_tensor(nwt, nwt, -0.5, nrm, Alu.mult, Alu.mult)
                        # r = (1.5 + nwt) * r
                        nc.vector.scalar_tensor_tensor(rno, nwt, 1.5, rno, Alu.add, Alu.mult)
                    dirt = work.tile([P, R, NN, 3], f32, tag="sq")
                    rno_b = rno.rearrange("p (r n) -> p r n", r=R)[:, :, :, None] \
                               .broadcast_to((P, R, NN, 3))
                    nc.vector.tensor_mul(dirt, pos_ap, rno_b)

                    # ---- cos[n,m] = sum_d dir[n,d]*cam[m,d] ----
                    ts = []
                    for d in range(3):
                        td = work.tile([P, R, NN, MM], f32, tag=f"t{d}")
                        dir_d = dirt[:, :, :, d:d + 1].broadcast_to((P, R, NN, MM))
                        cam_d = ct[:, :, None, :, d].broadcast_to((P, R, NN, MM))
                        _eng(nc, cfg, f"t{d}").tensor_mul(td, dir_d, cam_d)
                        ts.append(td)
                    cos = ts[0]
                    _eng(nc, cfg, "add1").tensor_add(cos, cos, ts[1])
                    _eng(nc, cfg, "add2").tensor_add(cos, cos, ts[2])

                    # ---- prob = exp(cos) / sum(exp(cos)) ----
                    prob = io.tile([P, R, NN, MM], f32, tag="prob")
                    nc.scalar.activation(out=prob, in_=cos, func=Act.Exp)
                    sume = work.tile([P, R * NN], f32, tag="sume")
                    nc.vector.reduce_sum(out=sume, in_=prob.rearrange("p r n m -> p (r n) m"), axis=AxX)
                    rsum = work.tile([P, R * NN], f32, tag="rsum")
                    nc.vector.reciprocal(rsum, sume)
                    rsum_b = rsum.rearrange("p (r n) -> p r n", r=R)[:, :, :, None] \
                                 .broadcast_to((P, R, NN, MM))
                    _eng(nc, cfg, "prob").tensor_mul(prob, prob, rsum_b)
                    nc.sync.dma_start(out=prob_v[t], in_=prob)

                    # ---- argmax (first-max, tie-exact) ----
                    mx = work.tile([P, R * NN], f32, tag="mx")
                    nc.vector.reduce_max(out=mx, in_=cos.rearrange("p r n m -> p (r n) m"), axis=AxX)
                    # eq/eqw in bf16: {0,1} and 0..15 are exact in bf16 and
                    # the all-16-bit multiply runs in the DVE 2x_1P mode.
                    eq = work.tile([P, R, NN, MM], mybir.dt.bfloat16, tag="eq")
                    mx_b = mx.rearrange("p (r n) -> p r n", r=R)[:, :, :, None] \
                             .broadcast_to((P, R, NN, MM))
                    _eng(nc, cfg, "eq").tensor_tensor(eq, cos, mx_b, Alu.is_equal)
                    wt_b = wt16[:, None, None, :].broadcast_to((P, R, NN, MM))
                    _eng(nc, cfg, "eqw").tensor_mul(eq, eq, wt_b)
                    wm = work.tile([P, R * NN], f32, tag="wm")
                    nc.vector.reduce_max(out=wm, in_=eq.rearrange("p r n m -> p (r n) m"), axis=AxX)
                    idxf = work.tile([P, R * NN], f32, tag="idxf")
                    nc.vector.tensor_scalar(idxf, wm, -1.0, 15.0, Alu.mult, Alu.add)

                    # ---- match_cam = sum_m onehot(idx)[m] * cam[m,:] ----
                    oh = work.tile([P, R, NN, MM], f32, tag="eqx")
                    it_b = it[:, None, None, :].broadcast_to((P, R, NN, MM))
                    idx_b = idxf.rearrange("p (r n) -> p r n", r=R)[:, :, :, None] \
                                .broadcast_to((P, R, NN, MM))
                    _eng(nc, cfg, "oh").tensor_tensor(oh, it_b, idx_b, Alu.is_equal)
                    match = work.tile([P, R, NN, 3], f32, tag="match")
                    for d in range(3):
                        tmpd = work.tile([P, R, NN, MM], f32, tag="tmpd")
                        camT_d = camT[:, :, None, d, :].broadcast_to((P, R, NN, MM))
                        _eng(nc, cfg, "tmp").tensor_mul(tmpd, oh, camT_d)
                        nc.vector.reduce_sum(
                            out=match[:, :, :, d],
                            in_=tmpd.rearrange("p r n m -> p (r n) m"), axis=AxX)

                    # ---- valid + blends ----
                    valid = work.tile([P, R * NN], f32, tag="valid")
                    nc.vector.tensor_single_scalar(valid, mx, 0.99, Alu.is_gt)

                    posx = io.tile([P, R, NN, 3], f32, tag="posx")
                    dis_b = dis_ap.broadcast_to((P, R, NN, 3))
                    if cfg.get("pospred"):
                        # pos = dis*match, overwritten with prior_pos where invalid
                        inval = work.tile([P, R * NN], mybir.dt.uint8, tag="nrm")
                        nc.vector.tensor_single_scalar(inval, mx, 0.99, Alu.is_le)
                        nc.vector.tensor_mul(posx, match, dis_b)
                        inval_b3 = inval[:, :, None].broadcast_to((P, R * NN, 3))
                        nc.vector.copy_predicated(
                            posx.rearrange("p r n d -> p (r n) d"), inval_b3,
                            pos_ap.rearrange("p r n d -> p (r n) d"))
                    else:
                        # pos = pp + valid*(dis*match - pp)
                        _eng(nc, cfg, "posml").tensor_mul(posx, match, dis_b)
                        _eng(nc, cfg, "possub").tensor_sub(posx, posx, pos_ap)
                        valid_b3 = valid.rearrange("p (r n) -> p r n", r=R)[:, :, :, None] \
                                        .broadcast_to((P, R, NN, 3))
                        _eng(nc, cfg, "posmv").tensor_mul(posx, posx, valid_b3)
                        _eng(nc, cfg, "posadd").tensor_add(posx, posx, pos_ap)
                    nc.sync.dma_start(out=pos_v[t], in_=posx)

                    # cov = ((1-mx)*100 clipped to [0.01,10]; 10 when invalid)
                    cov0 = work.tile([P, R * NN], f32, tag="cov0")
                    nc.vector.tensor_scalar(cov0, mx, -1.0, 1.0, Alu.mult, Alu.add)
                    nc.vector.tensor_scalar(cov0, cov0, 100.0, 0.01, Alu.mult, Alu.max)
                    v10 = work.tile([P, R * NN], f32, tag="v10")
                    nc.vector.tensor_scalar(v10, mx, 0.99, MAX_COV, Alu.is_le, Alu.mult)
                    cov = io.tile([P, R * NN], f32, tag="cov")
                    # cov = max(min(cov0, 10), v10)
                    nc.vector.scalar_tensor_tensor(cov, cov0, MAX_COV, v10, Alu.min, Alu.max)
                    nc.sync.dma_start(out=cov_v[t], in_=cov)

                    # indices = valid*(idx+1) - 1
                    indf = work.tile([P, R * NN], f32, tag="indf")
                    nc.vector.scalar_tensor_tensor(indf, idxf, 1.0, valid, Alu.add, Alu.mult)
                    nc.vector.tensor_scalar_sub(indf, indf, 1.0)
                    ind = io.tile([P, R * NN], mybir.dt.int32, tag="ind")
                    nc.vector.tensor_copy(ind, indf)
                    nc.sync.dma_start(out=ind_v[t], in_=ind)

    nc.compile()
    return nc, rows_pad


def _get_module():
    if "nc" not in _CACHE:
        _CACHE["nc"] = _build_module()
    return _CACHE["nc"]


def _make_in_maps(others_feat, others_cam, rows_pad):
    feat = np.ascontiguousarray(others_feat, dtype=np.float32).reshape(BSN, NN, 8)
    cam = np.ascontiguousarray(others_cam, dtype=np.float32).reshape(BSN, MM, 3)
    wdesc_np = np.arange(MM - 1, -1, -1, dtype=np.float32)
    iota_np = np.arange(MM, dtype=np.float32)
    in_maps = []
    for c in range(NCORES):
        fpad = np.zeros((rows_pad, NN, 8), dtype=np.float32)
        cpad = np.zeros((rows_pad, MM, 3), dtype=np.float32)
        fpad[:ROWS_CORE] = feat[c * ROWS_CORE:(c + 1) * ROWS_CORE]
        cpad[:ROWS_CORE] = cam[c * ROWS_CORE:(c + 1) * ROWS_CORE]
        in_maps.append({
            "feat": fpad.reshape(rows_pad * NN, 8),
            "cam": cpad.reshape(rows_pad * MM, 3),
            "wdesc": wdesc_np,
            "iota": iota_np,
        })
    return in_maps


def _gather(res_list, rows_pad):
    prob = np.concatenate([r["prob_o"][:ROWS_CORE] for r in res_list], axis=0)
    pos = np.concatenate([r["pos_o"][:ROWS_CORE] for r in res_list], axis=0)
    cov = np.concatenate([r["cov_o"][:ROWS_CORE, :, None] for r in res_list], axis=0)
    ind = np.concatenate([r["ind_o"][:ROWS_CORE, :, None] for r in res_list], axis=0)
    sco = np.concatenate(
        [r["sco_o"].reshape(rows_pad, NN + 1, MM + 1)[:ROWS_CORE] for r in res_list],
        axis=0)
    return (prob, pos, cov.astype(np.float32), sco, ind.astype(np.int32))


def run(others_feat, others_cam):
    nc, rows_pad = _get_module()
    in_maps = _make_in_maps(others_feat, others_cam, rows_pad)
    res = run_bass_kernel_spmd(nc, in_maps, core_ids=list(range(NCORES)))
    return _gather(res.results, rows_pad), res.exec_time_ns


def kernel(others_feat, others_cam):
    out, _ = run(others_feat, others_cam)
    return out


def _pjrt_exec(nc, in_maps, iters=5):
    """Mirror bass2jax's sharded execution without donation; return
    (per-core results, list of wall times for repeated executions)."""
    import jax
    from jax.sharding import Mesh, PartitionSpec, NamedSharding
    from jax.experimental.shard_map import shard_map
    import concourse.mybir as mb
    from concourse import bass2jax as b2j
    import time

    b2j.install_neuronx_cc_hook()
    partition_name = nc.partition_id_tensor.name if nc.partition_id_tensor else None
    in_names, out_names, out_avals, zero_outs = [], [], [], []
    for alloc in nc.m.functions[0].allocations:
        if not isinstance(alloc, mb.MemoryLocationSet):
            continue
        name = alloc.memorylocations[0].name
        if alloc.kind == "ExternalInput":
            if name != partition_name:
                in_names.append(name)
        elif alloc.kind == "ExternalOutput":
            out_names.append(name)
            shape = tuple(alloc.tensor_shape)
            dtype = mb.dt.np(alloc.dtype)
            out_avals.append(jax.core.ShapedArray(shape, dtype))
            zero_outs.append(np.zeros(shape, dtype))
    n_params = len(in_names)
    all_names = in_names + out_names
    if partition_name is not None:
        all_names = all_names + [partition_name]

    def _body(*args):
        operands = list(args)
        if partition_name is not None:
            operands.append(b2j.partition_id_tensor())
        outs = b2j._bass_exec_p.bind(
            *operands,
            out_avals=tuple(out_avals),
            in_names=tuple(all_names),
            out_names=tuple(out_names),
            lowering_input_output_aliases=(),
            sim_require_finite=True,
            sim_require_nnan=True,
            nc=nc,
        )
        return tuple(outs)

    devices = jax.devices()[:NCORES]
    mesh = Mesh(np.asarray(devices), ("core",))
    sh = NamedSharding(mesh, PartitionSpec("core"))
    in_specs = (PartitionSpec("core"),) * (n_params + len(out_names))
    out_specs = (PartitionSpec("core"),) * len(out_names)
    fn = jax.jit(shard_map(_body, mesh=mesh, in_specs=in_specs,
                           out_specs=out_specs, check_rep=False),
                 keep_unused=True)

    concat_in = [
        jax.device_put(
            np.concatenate([np.asarray(m[nm]) for m in in_maps], axis=0), sh)
        for nm in in_names
    ]
    concat_zeros = [
        jax.device_put(np.zeros((NCORES * z.shape[0], *z.shape[1:]), z.dtype), sh)
        for z in zero_outs
    ]

    out_arrs = fn(*concat_in, *concat_zeros)
    jax.block_until_ready(out_arrs)
    times = []
    for _ in range(iters):
        t0 = time.perf_counter()
        o = fn(*concat_in, *concat_zeros)
        jax.block_until_ready(o)
        times.append(time.perf_counter() - t0)

    res = [
        {name: np.asarray(out_arrs[i]).reshape(NCORES, *out_avals[i].shape)[c]
         for i, name in enumerate(out_names)}
        for c in range(NCORES)
    ]
    return res, times


def run_timed(others_feat, others_cam, iters=5):
    nc, rows_pad = _get_module()
    in_maps = _make_in_maps(others_feat, others_cam, rows_pad)
    res, times = _pjrt_exec(nc, in_maps, iters=iters)
    wall_ns = int(min(times) * 1e9)
    return _gather(res, rows_pad), wall_ns, times


def sim_exec_ns():
    """Cost-model (TimelineSim) per-core exec estimate — used when no
    hardware profiling (NTFF) is available in the container."""
    from concourse.timeline_sim import TimelineSim
    nc, _ = _get_module()
    ts = TimelineSim(nc, trace=False, no_exec=True)
    ts.simulate()
    return int(ts.time)
